# revision 20
# baseline (speedup 1.0000x reference)
"""Trainium2 Bass kernel for nn_AuxiliaryYFixed (segment_reduce).

Computes, for z_ins [N,128], sorted bag_idx [N], W [1,128], b [1]:
    loc = z_ins @ W.T + b                      -> [N, 1]
    M[s] = max(loc[i] for bag_idx[i]==s) or 0  -> [B, 1]
returning (M, loc) like the jax reference.

Strategy (8 NeuronCores, data-parallel over N):
  - Each core gets a contiguous shard of 253,952 rows (= 128 partitions x
    L=1984), overlapping 3,952 rows into the next shard (duplicates are
    harmless for max; loc is written only for the owned 250,000 rows).
  - On-chip, partition p owns local rows [p*L, (p+1)*L): DVE computes
    z * W (elementwise, W replicated) and a grouped reduce -> loc lands
    directly in [128, L] "scan layout".
  - Segmented running max via tensor_tensor_scan (op0=add with 0/-1e30
    boundary flags, op1=max), stitched across the 128 partition rows with a
    tiny transposed second scan.
  - Per-segment extraction: the last position of each segment holds the full
    segment max; positions are compacted 64:1 per group (valid because the
    minimum segment length >= 64) and scattered to a [B] partial-max array
    with one small indirect DMA (out-of-range offsets are skipped).
  - Host combines: concat loc shards; elementwise max of the 8 partial [B]
    arrays; untouched (empty) bags map to 0.0.
"""

import os
import sys

import numpy as np

for _p in ("/opt/trn_rl_repo", "/root/.axon_site/_ro/trn_rl_repo"):
    if os.path.isdir(_p) and _p not in sys.path:
        sys.path.insert(0, _p)

import concourse.bacc as bacc
import concourse.bass as bass
import concourse.mybir as mybir
import concourse.tile as tile
from concourse.bass import IndirectOffsetOnAxis
from concourse.tile import add_dep_helper

F32 = mybir.dt.float32
I32 = mybir.dt.int32
AX = mybir.AxisListType
OP = mybir.AluOpType

# Problem geometry (hardcoded per the harness contract).
N = 2_000_000
D = 128
B = 16384
NCORES = 8
P = 128
SHARD = N // NCORES          # 250,000 owned rows per core
L = 1984                     # free length per partition (31*64, 62*32)
NC_PAD = P * L               # 253,952 rows processed per core
JB = 32                      # j-columns per DMA/compute block (2 MiB blocks)
NJ = L // JB                 # 62
G = 64                       # extraction group size (min segment length >= G)
NG = L // G                  # 31
NEG = 1.0e30
OWN_FULL_PARTS = SHARD // L           # 126 full partitions owned
OWN_TAIL = SHARD - OWN_FULL_PARTS * L  # 16 leftover elements in partition 126


def build_program(finalize=True):
    nc = bacc.Bacc("TRN2", target_bir_lowering=False, debug=False)

    z_d = nc.dram_tensor("z", [NC_PAD, D], F32, kind="ExternalInput")
    idx_d = nc.dram_tensor("idxf", [NC_PAD], F32, kind="ExternalInput")
    wrep_d = nc.dram_tensor("wrepw", [P, JB * D], F32, kind="ExternalInput")
    brep_d = nc.dram_tensor("brep", [P, 1], F32, kind="ExternalInput")
    eye_d = nc.dram_tensor("eye", [P, P], F32, kind="ExternalInput")
    nextv_d = nc.dram_tensor("nextv", [1, 1], F32, kind="ExternalInput")

    loc_d = nc.dram_tensor("loc_out", [SHARD], F32, kind="ExternalOutput")
    # per-bag partial maxima; initialized to -1e30 on device, written only at
    # the last position of each segment present in the shard
    mc_d = nc.dram_tensor("mc_out", [B], F32, kind="ExternalOutput")

    z_v = z_d[:].rearrange("(p l) d -> p l d", p=P)
    idx_v = idx_d[:].rearrange("(p l) -> p l", p=P)

    with tile.TileContext(nc) as tc:
        with (
            tc.tile_pool(name="zp", bufs=3) as zp,
            tc.tile_pool(name="prodp", bufs=2) as prodp,
            tc.tile_pool(name="cst", bufs=1) as cst,
            tc.tile_pool(name="main", bufs=1) as main,
            tc.tile_pool(name="wk", bufs=1) as wk,
            tc.tile_pool(name="sm", bufs=1) as sm,
            tc.tile_pool(name="ps", bufs=2, space="PSUM") as ps,
        ):
            # ---- constants + idx ----
            wrep_t = cst.tile([P, JB * D], F32)
            nc.sync.dma_start(wrep_t[:], wrep_d[:])
            brep_t = cst.tile([P, 1], F32)
            nc.sync.dma_start(brep_t[:], brep_d[:])
            eye_t = cst.tile([P, P], F32)
            nc.sync.dma_start(eye_t[:], eye_d[:])
            nextv_t = cst.tile([1, 1], F32)
            nc.sync.dma_start(nextv_t[:], nextv_d[:])
            idx_t = main.tile([P, L], F32)
            nc.sync.dma_start(idx_t[:], idx_v)

            loc_t = main.tile([P, L], F32)

            # ---- phase A: loc = sum_d z*w, landing in scan layout ----
            with nc.named_scope("proj"):
                for t in range(NJ):
                    zt = zp.tile([P, JB * D], F32)
                    nc.sync.dma_start(zt[:], z_v[:, t * JB:(t + 1) * JB, :])
                    pr = prodp.tile([P, JB * D], F32)
                    nc.vector.tensor_tensor(pr[:], zt[:], wrep_t[:], op=OP.mult)
                    nc.vector.tensor_reduce(
                        loc_t[:, t * JB:(t + 1) * JB],
                        pr[:].rearrange("p (j d) -> p j d", d=D),
                        axis=AX.X,
                        op=OP.add,
                    )
                nc.vector.tensor_scalar(
                    loc_t[:], loc_t[:], brep_t[:, 0:1], None, op0=OP.add
                )

            # write owned loc rows out early (overlaps with the scan below)
            nc.sync.dma_start(
                loc_d[0:OWN_FULL_PARTS * L].rearrange("(p l) -> p l", p=OWN_FULL_PARTS),
                loc_t[0:OWN_FULL_PARTS, :],
            )
            nc.sync.dma_start(
                loc_d[OWN_FULL_PARTS * L:SHARD],
                loc_t[OWN_FULL_PARTS:OWN_FULL_PARTS + 1, 0:OWN_TAIL],
            )

            with nc.named_scope("scan"):
                # ---- phase B: segmented prefix max along each partition row ----
                bnd = wk.tile([P, L], F32, tag="w1")
                nc.vector.tensor_tensor(
                    bnd[:, 1:L], idx_t[:, 1:L], idx_t[:, 0:L - 1], op=OP.is_equal
                )
                nc.vector.tensor_scalar(
                    bnd[:, 1:L], bnd[:, 1:L], 1.0, NEG, op0=OP.subtract, op1=OP.mult
                )
                nc.vector.memset(bnd[:, 0:1], -NEG)
                m_t = wk.tile([P, L], F32, tag="w2")
                nc.vector.tensor_tensor_scan(
                    m_t[:], bnd[:], loc_t[:], initial=-NEG, op0=OP.add, op1=OP.max
                )

                # ---- phase C: stitch across partition rows ----
                # transpose per-row tail/first/last into [1,128] rows at partition 0
                pt = ps.tile([1, 3 * P], F32)
                nc.tensor.matmul(
                    out=pt[:, 0:P], lhsT=m_t[:, L - 1:L], rhs=eye_t[:],
                    start=True, stop=True,
                )
                nc.tensor.matmul(
                    out=pt[:, P:2 * P], lhsT=idx_t[:, 0:1], rhs=eye_t[:],
                    start=True, stop=True,
                )
                nc.tensor.matmul(
                    out=pt[:, 2 * P:3 * P], lhsT=idx_t[:, L - 1:L], rhs=eye_t[:],
                    start=True, stop=True,
                )
                tails = sm.tile([1, P], F32)
                firsts = sm.tile([1, P], F32)
                lasts = sm.tile([1, P], F32)
                nc.vector.tensor_copy(tails[:], pt[:, 0:P])
                nc.vector.tensor_copy(firsts[:], pt[:, P:2 * P])
                nc.vector.tensor_copy(lasts[:], pt[:, 2 * P:3 * P])

                cont = sm.tile([1, P], F32)
                nc.vector.memset(cont[:, 0:1], 0.0)
                nc.vector.tensor_tensor(
                    cont[:, 1:P], firsts[:, 1:P], lasts[:, 0:P - 1], op=OP.is_equal
                )
                single = sm.tile([1, P], F32)
                nc.vector.tensor_tensor(single[:], firsts[:], lasts[:], op=OP.is_equal)
                g_t = sm.tile([1, P], F32)
                nc.vector.tensor_tensor(g_t[:], cont[:], single[:], op=OP.mult)
                nc.vector.tensor_scalar(
                    g_t[:], g_t[:], 1.0, NEG, op0=OP.subtract, op1=OP.mult
                )
                u_t = sm.tile([1, P], F32)
                nc.vector.tensor_tensor_scan(
                    u_t[:], g_t[:], tails[:], initial=-NEG, op0=OP.add, op1=OP.max
                )
                # carry[p] = cont[p] ? u[p-1] : -NEG
                carry_r = sm.tile([1, P], F32)
                nc.vector.memset(carry_r[:, 0:1], -NEG)
                nc.vector.tensor_tensor(
                    carry_r[:, 1:P], u_t[:, 0:P - 1], cont[:, 1:P], op=OP.mult
                )
                tmp = sm.tile([1, P], F32)
                nc.vector.tensor_scalar(
                    tmp[:, 1:P], cont[:, 1:P], 1.0, NEG, op0=OP.subtract, op1=OP.mult
                )
                nc.vector.tensor_tensor(
                    carry_r[:, 1:P], carry_r[:, 1:P], tmp[:, 1:P], op=OP.add
                )
                # is-last flag for each row's final position
                lastf_r = sm.tile([1, P], F32)
                nc.vector.tensor_tensor(
                    lastf_r[:, 0:P - 1], lasts[:, 0:P - 1], firsts[:, 1:P],
                    op=OP.not_equal,
                )
                nc.vector.tensor_tensor(
                    lastf_r[:, P - 1:P], lasts[:, P - 1:P], nextv_t[0:1, 0:1],
                    op=OP.not_equal,
                )
                # transpose both rows back to [128,1] columns
                pc = ps.tile([P, 2], F32)
                nc.tensor.matmul(
                    out=pc[:, 0:1], lhsT=carry_r[:], rhs=eye_t[0:1, 0:1],
                    start=True, stop=True,
                )
                nc.tensor.matmul(
                    out=pc[:, 1:2], lhsT=lastf_r[:], rhs=eye_t[0:1, 0:1],
                    start=True, stop=True,
                )
                carry_c = sm.tile([P, 1], F32)
                nc.vector.tensor_copy(carry_c[:], pc[:, 0:1])

                # ---- phase D: fixup -> F = full segment max at each position ----
                cand = wk.tile([P, L], F32, tag="w3")
                nc.vector.tensor_scalar(
                    cand[:], idx_t[:], idx_t[:, 0:1], None, op0=OP.is_equal
                )
                nc.vector.tensor_scalar(
                    cand[:], cand[:], 1.0, NEG, op0=OP.subtract, op1=OP.mult
                )
                nc.vector.tensor_scalar(
                    cand[:], cand[:], carry_c[:, 0:1], None, op0=OP.add
                )
                F_t = wk.tile([P, L], F32, tag="w1")  # reuses bnd slot
                nc.vector.tensor_tensor(F_t[:], m_t[:], cand[:], op=OP.max)

            with nc.named_scope("extract"):
                islast = wk.tile([P, L], F32, tag="w4")
                nc.vector.tensor_tensor(
                    islast[:, 0:L - 1], idx_t[:, 0:L - 1], idx_t[:, 1:L], op=OP.not_equal
                )
                nc.vector.tensor_copy(islast[:, L - 1:L], pc[:, 1:2])

                # V = islast*F + (islast-1)*NEG  (F where last, -NEG elsewhere)
                V_t = wk.tile([P, L], F32, tag="w2")  # reuses m slot
                nc.vector.tensor_scalar(
                    V_t[:], islast[:], 1.0, NEG, op0=OP.subtract, op1=OP.mult
                )
                t2_t = wk.tile([P, L], F32, tag="w3")  # reuses cand slot
                nc.vector.tensor_tensor(t2_t[:], islast[:], F_t[:], op=OP.mult)
                nc.vector.tensor_tensor(V_t[:], V_t[:], t2_t[:], op=OP.add)
                Vg = sm.tile([P, NG], F32)
                nc.vector.tensor_reduce(
                    Vg[:], V_t[:].rearrange("p (g k) -> p g k", k=G), axis=AX.X, op=OP.max
                )
                O_t = wk.tile([P, L], F32, tag="w3")  # reuses cand slot
                nc.vector.tensor_scalar(O_t[:], idx_t[:], 1.0, None, op0=OP.add)
                nc.vector.tensor_tensor(O_t[:], O_t[:], islast[:], op=OP.mult)
                nc.vector.tensor_scalar(O_t[:], O_t[:], 1.0, None, op0=OP.subtract)
                Og = sm.tile([P, NG], F32)
                nc.vector.tensor_reduce(
                    Og[:], O_t[:].rearrange("p (g k) -> p g k", k=G), axis=AX.X, op=OP.max
                )
                offs_f = sm.tile([P, NG], F32)
                nc.vector.tensor_scalar(
                    offs_f[:], Og[:], 0.0, 2.0e6, op0=OP.is_lt, op1=OP.mult
                )
                nc.vector.tensor_tensor(offs_f[:], offs_f[:], Og[:], op=OP.add)
                offs_i = sm.tile([P, NG], I32)
                nc.vector.tensor_copy(offs_i[:], offs_f[:])

                # init partial maxima to -NEG, then scatter (one element per
                # offset; explicit dep — Tile doesn't order raw DRAM access
                # and the DMA rings drain concurrently)
                neg_t = sm.tile([P, B // P], F32)
                nc.vector.memset(neg_t[:], -NEG)
                init_i = nc.gpsimd.dma_start(
                    mc_d[:].rearrange("(p q) -> p q", p=P), neg_t[:]
                )
                # one indirect DMA per group column: the [128,1]-indices +
                # [128,1]-rows form is the hardware-validated embedding
                # pattern; multi-index-per-partition offset APs scramble
                mc_v = mc_d[:].rearrange("(b one) -> b one", one=1)
                for g in range(NG):
                    scat_i = nc.gpsimd.indirect_dma_start(
                        out=mc_v,
                        out_offset=IndirectOffsetOnAxis(
                            ap=offs_i[:, g:g + 1], axis=0
                        ),
                        in_=Vg[:, g:g + 1],
                        in_offset=None,
                        bounds_check=B - 1,
                        oob_is_err=False,
                    )
                    add_dep_helper(
                        scat_i.ins, init_i.ins, sync=True,
                        reason="scatter waits for mc init completion",
                    )
    nc.compile()
    if finalize:
        nc.finalize()
    return nc


_PROGRAM = None


def _get_program():
    global _PROGRAM
    if _PROGRAM is None:
        _PROGRAM = build_program()
    return _PROGRAM


def make_in_maps(z_ins, bag_idx, W, b):
    z = np.asarray(z_ins, dtype=np.float32)
    idxf = np.asarray(bag_idx).astype(np.float32)
    Wf = np.asarray(W, dtype=np.float32).reshape(1, D)
    bf = np.asarray(b, dtype=np.float32).reshape(-1)

    wrepw = np.tile(Wf, (P, JB)).astype(np.float32)          # [128, JB*128]
    brep = np.full((P, 1), bf[0], dtype=np.float32)
    eye = np.eye(P, dtype=np.float32)

    pad = NC_PAD - SHARD                                      # 3,952
    # pad z rows for the last core: loc = -1e4*||W||^2 + b, far below any
    # real loc, and idx continues the final segment so no fake boundary.
    zpad = np.tile((-1.0e4 * Wf).astype(np.float32), (pad, 1))
    s_last = idxf[-1]

    in_maps = []
    for c in range(NCORES):
        s0 = c * SHARD
        if c < NCORES - 1:
            zc = z[s0:s0 + NC_PAD]
            ic = idxf[s0:s0 + NC_PAD]
            nv = np.array([[idxf[s0 + NC_PAD]]], dtype=np.float32)
        else:
            zc = np.concatenate([z[s0:], zpad], axis=0)
            ic = np.concatenate([idxf[s0:], np.full(pad, s_last, np.float32)])
            nv = np.array([[-1.0]], dtype=np.float32)
        in_maps.append({
            "z": np.ascontiguousarray(zc),
            "idxf": np.ascontiguousarray(ic),
            "wrepw": wrepw,
            "brep": brep,
            "eye": eye,
            "nextv": nv,
        })
    return in_maps


def combine_outputs(results):
    loc = np.concatenate([np.asarray(r["loc_out"]) for r in results])
    mcs = np.stack([np.asarray(r["mc_out"]) for r in results], axis=0)  # [C,B]
    M = np.max(mcs, axis=0)
    M = np.where(M < -1.0e29, np.float32(0.0), M).astype(np.float32)
    return M[:, None], loc[:, None].astype(np.float32)


def kernel(z_ins, bag_idx, W, b):
    from concourse.bass_utils import run_bass_kernel_spmd

    nc = _get_program()
    in_maps = make_in_maps(z_ins, bag_idx, W, b)
    res = run_bass_kernel_spmd(nc, in_maps, core_ids=list(range(NCORES)))
    return combine_outputs(res.results)
